# revision 1
# baseline (speedup 1.0000x reference)
"""GateGATLayer kernel for 8 Trainium2 NeuronCores.

Strategy (per sharding_hint): data-parallel over the batch axis.
B=8, N=1024, H=512, NH=8 heads -> one batch element per core, weights
replicated. Each core computes the full GAT layer for its batch:
  q,k,v = x @ W{q,k,v}.T ; masked multi-head attention over adj;
  sigmoid-gated residual combine with Wg, bg.
Inputs arrive FULL; we shard over cores with jax.pmap and the stacked
leading batch axis, then the pmap output (already [8, N, H]) IS the
full output. Falls back to a pure-numpy implementation if no (or
too few) accelerator devices are available.
"""

import numpy as np

B, N, H, NH = 8, 1024, 512, 8
DK = H // NH


def _numpy_impl(x, adj, Wq, Wk, Wv, Wg, bg):
    x = x.astype(np.float32)
    q = (x @ Wq.T).reshape(B, N, NH, DK)
    k = (x @ Wk.T).reshape(B, N, NH, DK)
    v = (x @ Wv.T).reshape(B, N, NH, DK)
    # scores: [b, nh, n, n]
    scores = np.einsum("bqhd,bkhd->bhqk", q, k) / np.sqrt(np.float32(DK))
    mask = (adj != 0)[:, None, :, :]
    neg = np.float32(-1e30)
    scores = np.where(mask, scores, neg)
    scores -= scores.max(axis=-1, keepdims=True)
    e = np.exp(scores)
    attn = e / e.sum(axis=-1, keepdims=True)
    c = np.einsum("bhqk,bkhd->bqhd", attn, v).reshape(B, N, H)
    gate = 1.0 / (1.0 + np.exp(-(np.concatenate([c, x], axis=2) @ Wg.T + bg)))
    return (gate * x + (1.0 - gate) * c).astype(np.float32)


def _jax_pmap_impl(x, adj, Wq, Wk, Wv, Wg, bg):
    import jax
    import jax.numpy as jnp
    from functools import partial

    devs = jax.devices()
    if len(devs) < B:
        raise RuntimeError(f"need {B} devices, have {len(devs)}")

    @partial(jax.pmap, devices=devs[:B])
    def per_core(x1, adj1, Wq, Wk, Wv, Wg, bg):
        # x1: [N, H], adj1: [N, N] — one batch element on this core.
        q = (x1 @ Wq.T).reshape(N, NH, DK)
        k = (x1 @ Wk.T).reshape(N, NH, DK)
        v = (x1 @ Wv.T).reshape(N, NH, DK)
        scores = jnp.einsum("qhd,khd->hqk", q, k) / jnp.sqrt(jnp.float32(DK))
        mask = (adj1 != 0)[None, :, :]
        scores = jnp.where(mask, scores, jnp.float32(-1e30))
        attn = jax.nn.softmax(scores, axis=-1)
        c = jnp.einsum("hqk,khd->qhd", attn, v).reshape(N, H)
        gate = jax.nn.sigmoid(jnp.concatenate([c, x1], axis=1) @ Wg.T + bg)
        return gate * x1 + (1.0 - gate) * c

    rep = lambda w: jnp.broadcast_to(jnp.asarray(w), (B,) + w.shape)
    out = per_core(
        jnp.asarray(x), jnp.asarray(adj), rep(Wq), rep(Wk), rep(Wv), rep(Wg), rep(bg)
    )
    return np.asarray(out, dtype=np.float32)


def kernel(x, adj, Wq, Wk, Wv, Wg, bg):
    x = np.asarray(x, dtype=np.float32)
    adj = np.asarray(adj)
    Wq = np.asarray(Wq, dtype=np.float32)
    Wk = np.asarray(Wk, dtype=np.float32)
    Wv = np.asarray(Wv, dtype=np.float32)
    Wg = np.asarray(Wg, dtype=np.float32)
    bg = np.asarray(bg, dtype=np.float32)
    try:
        return _jax_pmap_impl(x, adj, Wq, Wk, Wv, Wg, bg)
    except Exception:
        return _numpy_impl(x, adj, Wq, Wk, Wv, Wg, bg)


# revision 2
# speedup vs baseline: 1.1233x; 1.1233x over previous
"""GateGATLayer kernel for 8 Trainium2 NeuronCores.

Strategy (per sharding_hint): data-parallel over the batch axis.
B=8, N=1024, H=512, NH=8 heads -> one batch element per core, weights
replicated. Each core computes the full GAT layer for its batch:
  q,k,v = x @ W{q,k,v}.T ; masked multi-head attention over adj;
  sigmoid-gated residual combine with Wg, bg.
Inputs arrive FULL; we shard over cores with jax.pmap and the stacked
leading batch axis, then the pmap output (already [8, N, H]) IS the
full output. Falls back to a pure-numpy implementation if no (or
too few) accelerator devices are available.
"""

import numpy as np

B, N, H, NH = 8, 1024, 512, 8
DK = H // NH


def _numpy_impl(x, adj, Wq, Wk, Wv, Wg, bg):
    x = x.astype(np.float32)
    q = (x @ Wq.T).reshape(B, N, NH, DK)
    k = (x @ Wk.T).reshape(B, N, NH, DK)
    v = (x @ Wv.T).reshape(B, N, NH, DK)
    # scores: [b, nh, n, n]
    scores = np.einsum("bqhd,bkhd->bhqk", q, k) / np.sqrt(np.float32(DK))
    mask = (adj != 0)[:, None, :, :]
    neg = np.float32(-1e30)
    scores = np.where(mask, scores, neg)
    scores -= scores.max(axis=-1, keepdims=True)
    e = np.exp(scores)
    attn = e / e.sum(axis=-1, keepdims=True)
    c = np.einsum("bhqk,bkhd->bqhd", attn, v).reshape(B, N, H)
    gate = 1.0 / (1.0 + np.exp(-(np.concatenate([c, x], axis=2) @ Wg.T + bg)))
    return (gate * x + (1.0 - gate) * c).astype(np.float32)


def _jax_pmap_impl(x, adj, Wq, Wk, Wv, Wg, bg):
    import jax
    import jax.numpy as jnp
    from functools import partial

    devs = jax.devices()
    if len(devs) < B:
        raise RuntimeError(f"need {B} devices, have {len(devs)}")

    @partial(
        jax.pmap,
        devices=devs[:B],
        in_axes=(0, 0, None, None, None, None, None),
    )
    def per_core(x1, adj1, Wq, Wk, Wv, Wg, bg):
        # x1: [N, H], adj1: [N, N] int8 — one batch element on this core.
        q = (x1 @ Wq.T).reshape(N, NH, DK)
        k = (x1 @ Wk.T).reshape(N, NH, DK)
        v = (x1 @ Wv.T).reshape(N, NH, DK)
        scores = jnp.einsum("qhd,khd->hqk", q, k) / jnp.sqrt(jnp.float32(DK))
        mask = (adj1 != 0)[None, :, :]
        scores = jnp.where(mask, scores, jnp.float32(-1e30))
        attn = jax.nn.softmax(scores, axis=-1)
        c = jnp.einsum("hqk,khd->qhd", attn, v).reshape(N, H)
        gate = jax.nn.sigmoid(jnp.concatenate([c, x1], axis=1) @ Wg.T + bg)
        return gate * x1 + (1.0 - gate) * c

    adj8 = (adj != 0).astype(np.int8)  # 4x smaller host->device transfer
    out = per_core(
        jnp.asarray(x), jnp.asarray(adj8), jnp.asarray(Wq), jnp.asarray(Wk),
        jnp.asarray(Wv), jnp.asarray(Wg), jnp.asarray(bg),
    )
    return np.asarray(out, dtype=np.float32)


def kernel(x, adj, Wq, Wk, Wv, Wg, bg):
    x = np.asarray(x, dtype=np.float32)
    adj = np.asarray(adj)
    Wq = np.asarray(Wq, dtype=np.float32)
    Wk = np.asarray(Wk, dtype=np.float32)
    Wv = np.asarray(Wv, dtype=np.float32)
    Wg = np.asarray(Wg, dtype=np.float32)
    bg = np.asarray(bg, dtype=np.float32)
    try:
        return _jax_pmap_impl(x, adj, Wq, Wk, Wv, Wg, bg)
    except Exception:
        return _numpy_impl(x, adj, Wq, Wk, Wv, Wg, bg)
